# revision 1
# baseline (speedup 1.0000x reference)
"""Trainium2 Bass kernel for CommittorNetBP (pairwise min-image env sum + tiny MLP).

Algorithm (mathematically equivalent reformulation of the reference):

 1. Per-component wrapped squared displacement is periodic in dx with period
    L=10, so  wrap(dx)^2 ~= B0 + sum_n Bn cos(2*pi*n*dx/L)  (constrained
    least-squares fit, N=16 harmonics, accurate on |dx| <= L/4 which covers
    the cutoff RC = L/4).  Hence d2[i,j] = sum_k wrap2(dx_k) is an inner
    product of trig embeddings: one TensorEngine matmul per 128-row block.
 2. The envelope f(t) = exp(-t)*0.5*(1+cos(pi*sqrt(t)/RC)) (t=d2, zero for
    t>=RC^2) is approximated by  w0 + sum_r w_r e^{-a_r t}  -> per-pair work
    is only Exp activations (single ACT table set; no sqrt/cos chain, no
    masking).  The constant w0 sums to 512*w0 per row and is folded into the
    MLP bias together with the diagonal correction (f~(0)=1 exactly via
    sum w = 1 and wrap2~(0)=0): b1' = b1 + (512*w0 - 1) * W1 @ ones.
 3. Row sums sum_j e_r[i,j] run on TensorE (ones-matmul, fp32r moving
    operand) and/or VectorE (reduce_sum), per REDUCE_ON[r].
 4. MLP: h = relu(inputt @ W1.T + b1'), out = sigmoid(h @ W2.T) computed as
    0.5 + 0.5*tanh(z/2) (tanh shares the exp ACT table set).

Sharding: pure data parallel, batch 128 -> 8 cores x 16.
"""

import numpy as np

# ---------------------------------------------------------------- constants
L = 10.0
RC = 2.5
PI = float(np.pi)
NP = 512
BTOT = 128
NCORES = 8
BLOC = BTOT // NCORES  # 16
NH = 16                # harmonics
K = 6 * NH + 1         # 97 embedding rows
NUM_NODES = 256

# wrap2(theta) ~= sum_n B[n] cos(n theta) (see fit.py)
B_HARM = [
    8.336507198660753, -10.134305777836879, 2.5283072633082164,
    -1.1207547738471013, 0.6351791173907125, -0.41237594667899846,
    0.28478810229590223, -0.20163605059415754, 0.15059719920404221,
    -0.12490354747428888, 0.11118898587488348, -0.09477489833163562,
    0.06985971056432684, -0.041620415059490684, 0.018837434788739185,
    -0.005869820105041354, 0.0009762178400180537,
]

# envelope fit: f(t) ~= W0 + sum_r WS[r] * exp(-ALPHAS[r] * t)
FIT = {
    2: dict(W0=0.00004956, ALPHAS=[1.206218, 1.161096],
            WS=[5.226685, -4.226734]),
    3: dict(W0=-0.00000457, ALPHAS=[0.962991, 1.039564, 1.162335],
            WS=[2.024749, -5.180925, 4.15618]),
}

# ------------------------------------------------------------- config
R = 2                      # number of exponential terms
W0 = FIT[R]["W0"]
ALPHAS = FIT[R]["ALPHAS"]
WS = FIT[R]["WS"]
MAIN_FP32R = False          # fp32r (1-pass) for the d2a matmul
REDUCE_ON = ["pe", "dve"] if R == 2 else ["pe", "pe", "dve"]

f32 = np.float32


def _host_constants():
    mt = np.zeros((4, K), f32)
    bcol = np.zeros((K, 1), f32)
    mt[3, 0] = 0.25            # const row: sin(2*pi*0.25) = 1
    bcol[0, 0] = 3.0 * B_HARM[0]
    col = 1
    for k in range(3):
        for n in range(1, NH + 1):
            mt[k, col] = n / L      # cos component (phase 0.25 turns)
            mt[3, col] = 0.25
            bcol[col, 0] = B_HARM[n]
            col += 1
            mt[k, col] = n / L      # sin component (phase 0)
            mt[3, col] = 0.0
            bcol[col, 0] = B_HARM[n]
            col += 1
    # stationary columns for the PE row-sum matmuls: sign(w_r) (exact in
    # any precision; |w_r| rides in the Exp bias as ln|w_r|)
    wcol = np.zeros((128, R), f32)
    for r in range(R):
        wcol[:, r] = 1.0 if WS[r] >= 0 else -1.0
    lnw = np.zeros((128, R), f32)
    for r in range(R):
        lnw[:, r] = np.log(abs(WS[r]))
    eye16 = np.eye(16, dtype=f32)
    return mt, bcol, wcol, lnw, eye16


_CACHE = {}


def _build_program():
    import concourse.bacc as bacc
    import concourse.mybir as mybir
    import concourse.tile as tile

    nc = bacc.Bacc("TRN2", target_bir_lowering=False, debug=False,
                   num_devices=NCORES)
    dt = mybir.dt
    AF = mybir.ActivationFunctionType
    ALU = mybir.AluOpType
    edt = dt.float32r if MAIN_FP32R else dt.float32
    n_pe = sum(1 for a in REDUCE_ON if a == "pe")

    xa_d = nc.declare_dram_parameter("xa", (4, BLOC * NP), dt.float32, isOutput=False)
    mt_d = nc.declare_dram_parameter("mt", (4, K), dt.float32, isOutput=False)
    bcol_d = nc.declare_dram_parameter("bcol", (K, 1), dt.float32, isOutput=False)
    wcol_d = nc.declare_dram_parameter("wcol", (128, R), dt.float32, isOutput=False)
    lnw_d = nc.declare_dram_parameter("lnw", (128, R), dt.float32, isOutput=False)
    w1t_d = nc.declare_dram_parameter("w1t", (NP, NUM_NODES), dt.float32, isOutput=False)
    b1p_d = nc.declare_dram_parameter("b1p", (1, NUM_NODES), dt.float32, isOutput=False)
    w2r_d = nc.declare_dram_parameter("w2r", (BLOC, NUM_NODES), dt.float32, isOutput=False)
    eye_d = nc.declare_dram_parameter("eye16", (16, 16), dt.float32, isOutput=False)
    y_d = nc.declare_dram_parameter("y", (BLOC, 1), dt.float32, isOutput=True)

    with tile.TileContext(nc) as tc:
        with tc.tile_pool(name="const", bufs=1) as cpool:
            xa_s = cpool.tile([4, BLOC * NP], dt.float32)
            nc.gpsimd.dma_start(xa_s[:], xa_d[:])
            mt_s = cpool.tile([4, K], dt.float32)
            nc.gpsimd.dma_start(mt_s[:], mt_d[:])
            bcol_s = cpool.tile([K, 1], dt.float32)
            nc.gpsimd.dma_start(bcol_s[:], bcol_d[:])
            wcol_s = cpool.tile([128, R], dt.float32r)
            nc.gpsimd.dma_start(wcol_s[:], wcol_d[:])
            lnw_s = cpool.tile([128, R], dt.float32)
            nc.gpsimd.dma_start(lnw_s[:], lnw_d[:])
            w1t_s = cpool.tile([128, 4 * NUM_NODES], dt.float32)
            for c in range(4):
                nc.gpsimd.dma_start(
                    w1t_s[:, c * NUM_NODES:(c + 1) * NUM_NODES],
                    w1t_d[c * 128:(c + 1) * 128, :])
            b1p_s = cpool.tile([1, NUM_NODES], dt.float32)
            nc.gpsimd.dma_start(b1p_s[:], b1p_d[:])
            w2r_s = cpool.tile([BLOC, NUM_NODES], dt.float32)
            nc.gpsimd.dma_start(w2r_s[:], w2r_d[:])
            eye_s = cpool.tile([16, 16], dt.float32)
            nc.gpsimd.dma_start(eye_s[:], eye_d[:])
            ones1_s = cpool.tile([1, BLOC], dt.float32)
            nc.gpsimd.memset(ones1_s[:], 1.0)

            # ---------------- phase 1: trig embeddings per batch ----------------
            with (
                tc.tile_pool(name="upsum", bufs=2, space="PSUM") as upsum,
                tc.tile_pool(name="ri", bufs=2) as ripool,
                tc.tile_pool(name="vv", bufs=2) as vpool,
                tc.tile_pool(name="E", bufs=BLOC) as epool,
                tc.tile_pool(name="Ew", bufs=BLOC) as ewpool,
            ):
                E_l, Ew_l = [], []
                for b in range(BLOC):
                    u = upsum.tile([K, NP], dt.float32)
                    nc.tensor.matmul(u[:], mt_s[:], xa_s[:, b * NP:(b + 1) * NP],
                                     start=True, stop=True)
                    ri = ripool.tile([K, NP], dt.int32)
                    nc.vector.tensor_copy(ri[:], u[:])          # round to nearest
                    v = vpool.tile([K, NP], dt.float32)
                    nc.vector.tensor_tensor(v[:], u[:], ri[:], ALU.subtract)
                    E = epool.tile([K, NP], edt, tag="E")
                    nc.scalar.activation(E[:], v[:], AF.Sin, scale=2.0 * PI)
                    Ew = ewpool.tile([K, NP], edt, tag="Ew")
                    nc.vector.tensor_scalar(Ew[:], E[:], bcol_s[:, 0:1], None, ALU.mult)
                    E_l.append(E)
                    Ew_l.append(Ew)

                # keep all Sin ops ahead of all Exp ops in the ACT stream
                # (sin and exp live in different ACT table sets).
                tc.no_sync_barrier()

                # ---------------- phase 2: pair blocks ----------------
                scopy = cpool.tile([BLOC, NP], dt.float32)
                with (
                    tc.tile_pool(name="acc", bufs=8) as accpool,
                    tc.tile_pool(name="tpsum", bufs=2, space="PSUM") as tpsum,
                    tc.tile_pool(name="spsum", bufs=2, space="PSUM") as spsum,
                    tc.tile_pool(name="ssb", bufs=2) as ssbpool,
                    tc.tile_pool(name="er", bufs=3) as erpool,
                ):
                    # dve-side accumulators: acc[jc][i] (i-th dve term)
                    dve_rs = [r for r in range(R) if REDUCE_ON[r] == "dve"]
                    pe_rs = [r for r in range(R) if REDUCE_ON[r] == "pe"]
                    acc = [[accpool.tile([128, BLOC], dt.float32,
                                         name=f"acc{jc}_{r}", tag=f"a{jc}_{r}")
                            for r in dve_rs] for jc in range(4)]
                    for b in range(BLOC):
                        srow = (spsum.tile([1, NP], dt.float32, tag="srow", name="srow")
                                if pe_rs else None)
                        n_acc = 4 * len(pe_rs)  # matmuls accumulating into srow
                        i_acc = 0
                        for g in range(2):
                            t = tpsum.tile([128, 2 * NP], dt.float32, tag="t")
                            for jj in range(2):
                                jc = 2 * g + jj
                                nc.tensor.matmul(
                                    t[:, jj * NP:(jj + 1) * NP],
                                    Ew_l[b][:, jc * 128:(jc + 1) * 128],
                                    E_l[b][:],
                                    start=True, stop=True)
                            for r in range(R):
                                er = erpool.tile([128, 2 * NP], dt.float32r,
                                                 tag="er")
                                nc.scalar.activation(
                                    er[:], t[:], AF.Exp, scale=-ALPHAS[r],
                                    bias=lnw_s[:, r:r + 1])
                                if REDUCE_ON[r] == "pe":
                                    # sum over partitions (== sum over j by
                                    # symmetry), w_r baked into the column
                                    for jj in range(2):
                                        nc.tensor.matmul(
                                            srow[:], wcol_s[:, r:r + 1],
                                            er[:, jj * NP:(jj + 1) * NP],
                                            start=(i_acc == 0),
                                            stop=(i_acc == n_acc - 1),
                                            skip_group_check=True)
                                        i_acc += 1
                                else:
                                    i_dve = dve_rs.index(r)
                                    for jj in range(2):
                                        jc = 2 * g + jj
                                        nc.vector.reduce_sum(
                                            acc[jc][i_dve][:, b:b + 1],
                                            er[:, jj * NP:(jj + 1) * NP],
                                            axis=mybir.AxisListType.X)
                        if pe_rs:
                            ssb = ssbpool.tile([1, NP], dt.float32, tag="ssb")
                            nc.vector.tensor_copy(ssb[:], srow[:])
                            nc.gpsimd.dma_start(scopy[b:b + 1, :], ssb[:])

                    # dve-side inputt chunks (already transposed layout);
                    # accs carry |w_r| from the exp bias, signs applied here
                    it_l = []
                    for jc in range(4):
                        if not dve_rs:
                            break
                        it = cpool.tile([128, BLOC], dt.float32, tag=f"it{jc}",
                                        name=f"it{jc}")
                        if len(dve_rs) == 1:
                            sgn = 1.0 if WS[dve_rs[0]] >= 0 else -1.0
                            nc.vector.tensor_scalar(it[:], acc[jc][0][:],
                                                    sgn, None, ALU.mult)
                        else:
                            op = (ALU.add if WS[dve_rs[1]] * WS[dve_rs[0]] >= 0
                                  else ALU.subtract)
                            nc.vector.tensor_tensor(it[:], acc[jc][0][:],
                                                    acc[jc][1][:], op)
                            if WS[dve_rs[0]] < 0:
                                nc.vector.tensor_scalar(it[:], it[:], -1.0,
                                                        None, ALU.mult)
                            for i in range(2, len(dve_rs)):
                                sop = (ALU.add if WS[dve_rs[i]] >= 0
                                       else ALU.subtract)
                                nc.vector.tensor_tensor(it[:], it[:],
                                                        acc[jc][i][:], sop)
                        it_l.append(it)

                # ---------------- phase 3: MLP tail ----------------
                with (
                    tc.tile_pool(name="trpsum", bufs=2, space="PSUM") as trpsum,
                    tc.tile_pool(name="hpsum", bufs=1, space="PSUM") as hpsum,
                    tc.tile_pool(name="tail", bufs=1) as tail,
                ):
                    n_pe_r = len([r for r in range(R) if REDUCE_ON[r] == "pe"])
                    mm_total = 4 * (1 if n_pe_r else 0) + (4 if it_l else 0) + 1
                    i_mm = 0
                    h = hpsum.tile([BLOC, NUM_NODES], dt.float32)
                    if n_pe_r:
                        # transpose PE-side rows [16,512] -> 4x [128,16]
                        for c in range(4):
                            tp = trpsum.tile([128, BLOC], dt.float32, tag="tp")
                            nc.tensor.transpose(
                                tp[:], scopy[:, c * 128:(c + 1) * 128], eye_s[:])
                            itp = tail.tile([128, BLOC], dt.float32,
                                            tag=f"itp{c}", name=f"itp{c}")
                            nc.vector.tensor_copy(itp[:], tp[:])
                            nc.tensor.matmul(
                                h[:], itp[:],
                                w1t_s[:, c * NUM_NODES:(c + 1) * NUM_NODES],
                                start=(i_mm == 0), stop=(i_mm == mm_total - 1),
                                skip_group_check=True)
                            i_mm += 1
                    for c in range(4):
                        if not it_l:
                            break
                        nc.tensor.matmul(
                            h[:], it_l[c][:],
                            w1t_s[:, c * NUM_NODES:(c + 1) * NUM_NODES],
                            start=(i_mm == 0), stop=(i_mm == mm_total - 1),
                            skip_group_check=True)
                        i_mm += 1
                    nc.tensor.matmul(h[:], ones1_s[:], b1p_s[:],
                                     start=False, stop=True,
                                     skip_group_check=True)
                    hr = tail.tile([BLOC, NUM_NODES], dt.float32)
                    nc.scalar.activation(hr[:], h[:], AF.Relu)
                    hw = tail.tile([BLOC, NUM_NODES], dt.float32)
                    nc.vector.tensor_tensor(hw[:], hr[:], w2r_s[:], ALU.mult)
                    z = tail.tile([BLOC, 1], dt.float32)
                    nc.vector.reduce_sum(z[:], hw[:], axis=mybir.AxisListType.X)
                    th = tail.tile([BLOC, 1], dt.float32)
                    nc.scalar.activation(th[:], z[:], AF.Tanh, scale=0.5)
                    ys = tail.tile([BLOC, 1], dt.float32)
                    nc.vector.tensor_scalar(ys[:], th[:], 0.5, 0.5,
                                            ALU.mult, ALU.add)
                    nc.gpsimd.dma_start(y_d[:], ys[:])

    nc.finalize()
    return nc


def _get_program():
    if "nc" not in _CACHE:
        _CACHE["nc"] = _build_program()
    return _CACHE["nc"]


def _make_in_maps(x, W1, b1, W2):
    mt, bcol, wcol, lnw, eye16 = _host_constants()
    W1 = np.asarray(W1, f32)
    w1t = np.ascontiguousarray(W1.T)
    b1p = (np.asarray(b1, f32)
           + (NP * f32(W0) - 1.0) * W1.sum(axis=1)).reshape(1, NUM_NODES).astype(f32)
    w2r = np.broadcast_to(np.asarray(W2, f32).reshape(1, NUM_NODES),
                          (BLOC, NUM_NODES)).copy()
    x = np.asarray(x, f32)
    in_maps = []
    for c in range(NCORES):
        xs = x[c * BLOC:(c + 1) * BLOC]                         # [16,512,3]
        xT = np.transpose(xs, (2, 0, 1)).reshape(3, BLOC * NP)  # [3,16*512]
        xa = np.concatenate([xT, np.ones((1, BLOC * NP), f32)], axis=0)
        in_maps.append({
            "xa": np.ascontiguousarray(xa),
            "mt": mt, "bcol": bcol, "wcol": wcol, "lnw": lnw,
            "w1t": w1t, "b1p": b1p, "w2r": w2r, "eye16": eye16,
        })
    return in_maps


def kernel(x, W1, b1, W2, _trace=False, _trace_kwargs=None):
    from concourse.bass_utils import run_bass_kernel_spmd

    nc = _get_program()
    in_maps = _make_in_maps(x, W1, b1, W2)
    res = run_bass_kernel_spmd(nc, in_maps, list(range(NCORES)),
                               trace=_trace, **(_trace_kwargs or {}))
    out = np.concatenate([res.results[c]["y"] for c in range(NCORES)], axis=0)
    if _trace:
        _CACHE["last_result"] = res
    return out.astype(f32)



# revision 13
# speedup vs baseline: 2.1280x; 2.1280x over previous
"""Trainium2 Bass kernel for CommittorNetBP (pairwise min-image env sum + tiny MLP).

Mathematically equivalent reformulation of the reference:

 1. Per-component wrapped squared displacement is periodic (L=10):
    wrap(dx)^2 ~= B0 + sum_n Bn cos(2*pi*n*dx/L)  (N=16 harmonics,
    weighted LS, exact at dx=0).  Via product-to-sum, d2[i,j] becomes an
    inner product of trig embeddings E[k, j] = sin(2*pi(n x_j/L + phase_k)).
    The HOST supplies uint16-quantized pre-wrapped phases so the device
    computes E with a single Sin activation per chunk (arg in [-pi, pi],
    inside the ACT table's accurate range) - no wrap ops, no phase matmul.
 2. Envelope f(t) = exp(-t)*0.5*(1+cos(pi*sqrt(t)/RC)) (t=d2, 0 beyond RC^2)
    fitted as  w0 + w1*e^{-a t} + w2*e^{-2a t}  (max fit err ~1e-3).  Only
    e^{-a t} needs the ACT engine; the square rides in a fused DVE
    scalar_tensor_tensor:  y = (e + w1/w2)*e,  with accum_out producing the
    per-particle row sums for free.  w2 is folded into W1 on the host.
 3. Pair symmetry: only upper-triangle 128-blocks are computed (10/16 of
    the work).  Diagonal blocks are internally symmetric, so row sums cover
    them; off-diagonal column sums run on the TensorEngine as ones-vector
    matmuls accumulating into a [16, 512] PSUM tile (one row per batch).
 4. MLP tail: h = relu(it @ (w2 W1).T + b1'), out = 0.5 + 0.5 tanh(z/2).

Sharding: pure data parallel, batch 128 -> 8 cores x 16.
"""

import numpy as np

# ---------------------------------------------------------------- constants
L = 10.0
PI = float(np.pi)
NP = 512
BTOT = 128
NCORES = 8
BLOC = BTOT // NCORES  # 16
NH = 16
K = 6 * NH             # 96 embedding rows (harmonics only; B0 in exp bias)
NUM_NODES = 256

# wrap2(dx) ~= B_HARM[0] + sum_n B_HARM[n] cos(2 pi n dx / L)  (see fit.py)
B_HARM = [
    8.33439917976705, -10.13262148690585, 2.5309960543177032,
    -1.1253735318191005, 0.6353410560854755, -0.4067952364402498,
    0.2795423738723759, -0.20515892530470042, 0.16010507643645575,
    -0.1281555455276066, 0.10007840117890257, -0.07991307904837286,
    0.07055404942212071, -0.06652343177790854, 0.05512886424596794,
    -0.03440721965376206, 0.012803401151499008,
]

# envelope fit: f(t) ~= EW0 + EW1 exp(-A t) + EW2 exp(-2 A t)
EW0 = 4.218244372734287e-05
EW1 = -0.057432602447565484
EW2 = 1.0583648509498493
A = 0.6827232177720551
CC = EW1 / EW2         # stt scalar: y = (e + CC) * e
B0C = 3.0 * B_HARM[0]  # constant part of d2a, folded into the exp bias
TCOMP = 0.0024         # fp32r truncation bias compensation on t

f32 = np.float32

# phase-2 packed t/er/y layout: (chunk I, start, width); j range of chunk I
# rows is [I*128, 512) so widths are 512, 384, 128, 256 packed tightly.
SEG = [(0, 0, 512), (1, 512, 384), (3, 896, 128), (2, 1024, 256)]
WTOT = 1280

_CACHE = {}


def _build_program():
    import concourse.bacc as bacc
    import concourse.mybir as mybir
    import concourse.tile as tile

    nc = bacc.Bacc("TRN2", target_bir_lowering=False, debug=False,
                   num_devices=NCORES)
    dt = mybir.dt
    AF = mybir.ActivationFunctionType
    ALU = mybir.AluOpType

    xh_d = nc.declare_dram_parameter("xh", (K, BLOC * NP), dt.uint16, isOutput=False)
    bcol_d = nc.declare_dram_parameter("bcol", (K, 1), dt.float32, isOutput=False)
    wcol_d = nc.declare_dram_parameter("wcol", (128, BLOC * BLOC), dt.bfloat16, isOutput=False)
    w1t_d = nc.declare_dram_parameter("w1t", (NP, NUM_NODES), dt.float32, isOutput=False)
    b1p_d = nc.declare_dram_parameter("b1p", (1, NUM_NODES), dt.float32, isOutput=False)
    w2r_d = nc.declare_dram_parameter("w2r", (BLOC, NUM_NODES), dt.float32, isOutput=False)
    eye_d = nc.declare_dram_parameter("eye16", (16, 16), dt.float32, isOutput=False)
    y_d = nc.declare_dram_parameter("y", (BLOC, 1), dt.float32, isOutput=True)

    NCH = 4                       # xh DMA / Sin chunks
    CW = BLOC * NP // NCH         # 2048 cols per chunk

    with tile.TileContext(nc) as tc:
        with (
            tc.tile_pool(name="const", bufs=1) as cpool,
            tc.tile_pool(name="srowp", bufs=1, space="PSUM") as srowp,
        ):
            xh_s = cpool.tile([K, BLOC * NP], dt.uint16)
            for ch in range(NCH):
                nc.gpsimd.dma_start(xh_s[:, ch * CW:(ch + 1) * CW],
                                    xh_d[:, ch * CW:(ch + 1) * CW])
            bcol_s = cpool.tile([K, 1], dt.float32)
            nc.gpsimd.dma_start(bcol_s[:], bcol_d[:])
            wcol_s = cpool.tile([128, BLOC * BLOC], dt.bfloat16)
            nc.gpsimd.dma_start(wcol_s[:], wcol_d[:])
            w1t_s = cpool.tile([128, 4 * NUM_NODES], dt.float32r)
            for c in range(4):
                nc.gpsimd.dma_start(
                    w1t_s[:, c * NUM_NODES:(c + 1) * NUM_NODES],
                    w1t_d[c * 128:(c + 1) * 128, :])
            b1p_s = cpool.tile([1, NUM_NODES], dt.float32)
            nc.gpsimd.dma_start(b1p_s[:], b1p_d[:])
            w2r_s = cpool.tile([BLOC, NUM_NODES], dt.float32)
            nc.gpsimd.dma_start(w2r_s[:], w2r_d[:])
            eye_s = cpool.tile([16, 16], dt.float32)
            nc.gpsimd.dma_start(eye_s[:], eye_d[:])
            ones1_s = cpool.tile([1, BLOC], dt.float32)
            nc.gpsimd.memset(ones1_s[:], 1.0)
            sbias_s = cpool.tile([K, 1], dt.float32)
            nc.gpsimd.memset(sbias_s[:], -PI)
            ebias_s = cpool.tile([128, 1], dt.float32)
            nc.gpsimd.memset(ebias_s[:], -A * (B0C - TCOMP))

            E_s = cpool.tile([K, BLOC * NP], dt.float32r, name="E")
            Ew_s = cpool.tile([K, BLOC * NP], dt.float32r, name="Ew")
            acc = [cpool.tile([128, BLOC], dt.float32, name=f"acc{i}")
                   for i in range(4)]
            # column sums accumulate here: row b = batch b, js 128..512
            srow = srowp.tile([BLOC, NP], dt.float32)

            # ---------------- phase 1: embeddings ----------------
            for ch in range(NCH):
                nc.scalar.activation(E_s[:, ch * CW:(ch + 1) * CW],
                                     xh_s[:, ch * CW:(ch + 1) * CW],
                                     AF.Sin, scale=2.0 * PI / 65536.0,
                                     bias=sbias_s[:, 0:1])
                nc.vector.tensor_scalar(Ew_s[:, ch * CW:(ch + 1) * CW],
                                        E_s[:, ch * CW:(ch + 1) * CW],
                                        bcol_s[:, 0:1], None, ALU.mult)

            tc.no_sync_barrier()

            # ---------------- phase 2: pair blocks ----------------
            with (
                tc.tile_pool(name="tpsum", bufs=2, space="PSUM") as tpsum,
                tc.tile_pool(name="er", bufs=2) as erpool,
                tc.tile_pool(name="yy", bufs=2) as ypool,
            ):
                for b in range(BLOC):
                    o = b * NP
                    t = tpsum.tile([128, WTOT], dt.float32, tag="t")
                    for I, s, w in SEG:
                        nc.tensor.matmul(
                            t[:, s:s + w],
                            Ew_s[:, o + I * 128:o + (I + 1) * 128],
                            E_s[:, o + I * 128:o + NP],
                            start=True, stop=True, skip_group_check=True)
                    er = erpool.tile([128, WTOT], dt.bfloat16, tag="er")
                    nc.scalar.activation(er[:], t[:], AF.Exp, scale=-A,
                                         bias=ebias_s[:, 0:1])
                    y = ypool.tile([128, WTOT], dt.bfloat16, tag="y")
                    for I, s, w in SEG:
                        nc.vector.scalar_tensor_tensor(
                            y[:, s:s + w], er[:, s:s + w], CC, er[:, s:s + w],
                            ALU.add, ALU.mult, accum_out=acc[I][:, b:b + 1])
                    i_cs = 0
                    for I, s, w in SEG:
                        if w <= 128:
                            continue  # diagonal-only chunk: no column sums
                        # wsel_b: ones in column b -> result lands in row b,
                        # zeros accumulate harmlessly into the other rows
                        nc.tensor.matmul(
                            srow[:, (I + 1) * 128:NP],
                            wcol_s[:, b * BLOC:(b + 1) * BLOC],
                            y[:, s + 128:s + w],
                            start=(b == 0 and i_cs == 0),
                            stop=(b == BLOC - 1 and i_cs == 2),
                            skip_group_check=True)
                        i_cs += 1

            # ---------------- phase 3: recombine + MLP ----------------
            with (
                tc.tile_pool(name="trpsum", bufs=2, space="PSUM") as trpsum,
                tc.tile_pool(name="hpsum", bufs=1, space="PSUM") as hpsum,
                tc.tile_pool(name="tail", bufs=1) as tail,
            ):
                scopy = tail.tile([BLOC, 3 * 128], dt.float32)
                nc.vector.tensor_copy(scopy[:], srow[:, 128:NP])
                it = [tail.tile([128, BLOC], dt.float32r, name=f"it{i}")
                      for i in range(4)]
                nc.vector.tensor_copy(it[0][:], acc[0][:])
                for c in range(1, 4):
                    tp = trpsum.tile([128, BLOC], dt.float32, tag="tp")
                    nc.tensor.transpose(
                        tp[:], scopy[:, (c - 1) * 128:c * 128], eye_s[:])
                    nc.vector.tensor_tensor(it[c][:], tp[:], acc[c][:],
                                            ALU.add)
                h = hpsum.tile([BLOC, NUM_NODES], dt.float32)
                for c in range(4):
                    nc.tensor.matmul(
                        h[:], it[c][:],
                        w1t_s[:, c * NUM_NODES:(c + 1) * NUM_NODES],
                        start=(c == 0), stop=False, skip_group_check=True)
                nc.tensor.matmul(h[:], ones1_s[:], b1p_s[:],
                                 start=False, stop=True, skip_group_check=True)
                hr = tail.tile([BLOC, NUM_NODES], dt.float32)
                nc.scalar.activation(hr[:], h[:], AF.Relu)
                z = tail.tile([BLOC, 1], dt.float32)
                hw = tail.tile([BLOC, NUM_NODES], dt.float32)
                nc.vector.scalar_tensor_tensor(
                    hw[:], hr[:], 1.0, w2r_s[:], ALU.mult, ALU.mult,
                    accum_out=z[:])
                th = tail.tile([BLOC, 1], dt.float32)
                nc.scalar.activation(th[:], z[:], AF.Tanh, scale=0.5)
                ys = tail.tile([BLOC, 1], dt.float32)
                nc.vector.tensor_scalar(ys[:], th[:], 0.5, 0.5,
                                        ALU.mult, ALU.add)
                nc.gpsimd.dma_start(y_d[:], ys[:])

    nc.finalize()
    return nc


def _get_program():
    if "nc" not in _CACHE:
        _CACHE["nc"] = _build_program()
    return _CACHE["nc"]


def _host_xh(xs):
    """xs [BLOC, NP, 3] -> uint16 phase rows [K, BLOC*NP]."""
    nb = xs.shape[0]
    xh = np.empty((K, nb, NP), np.float64)
    xT = xs.astype(np.float64) / L          # [nb, NP, 3]
    col = 0
    for k in range(3):
        xk = xT[:, :, k]
        for n in range(1, NH + 1):
            base = n * xk
            xh[col] = np.mod(base + 0.25, 1.0)      # cos row (phase .25)
            xh[col + 1] = np.mod(base, 1.0)          # sin row
            col += 2
    q = np.round((xh + 0.5) * 65536.0).astype(np.int64) % 65536
    return q.astype(np.uint16).reshape(K, nb * NP)


def _make_in_maps(x, W1, b1, W2):
    try:
        import ml_dtypes
        bf16 = ml_dtypes.bfloat16
    except ImportError:
        bf16 = None
    bcol = np.zeros((K, 1), f32)
    col = 0
    for k in range(3):
        for n in range(1, NH + 1):
            bcol[col, 0] = B_HARM[n]
            bcol[col + 1, 0] = B_HARM[n]
            col += 2
    wcol = np.tile(np.eye(BLOC, dtype=f32).reshape(1, BLOC * BLOC),
                   (128, 1))
    wcol = np.ascontiguousarray(wcol)
    wcol = wcol.astype(bf16) if bf16 is not None else wcol
    W1 = np.asarray(W1, f32)
    w1t = np.ascontiguousarray((f32(EW2) * W1).T).astype(f32)
    # b1' = b1 + (NP*w0 - (w0 + w1 + w2)) * (W1 @ ones)
    corr = f32(NP * EW0 - (EW0 + EW1 + EW2))
    b1p = (np.asarray(b1, f32) + corr * W1.sum(axis=1)).reshape(1, NUM_NODES).astype(f32)
    w2r = np.broadcast_to(np.asarray(W2, f32).reshape(1, NUM_NODES),
                          (BLOC, NUM_NODES)).copy()
    eye16 = np.eye(16, dtype=f32)
    x = np.asarray(x, f32)
    in_maps = []
    for c in range(NCORES):
        xs = x[c * BLOC:(c + 1) * BLOC]
        in_maps.append({
            "xh": _host_xh(xs),
            "bcol": bcol, "wcol": wcol,
            "w1t": w1t, "b1p": b1p, "w2r": w2r, "eye16": eye16,
        })
    return in_maps


def kernel(x, W1, b1, W2, _trace=False, _trace_kwargs=None):
    from concourse.bass_utils import run_bass_kernel_spmd

    nc = _get_program()
    in_maps = _make_in_maps(x, W1, b1, W2)
    res = run_bass_kernel_spmd(nc, in_maps, list(range(NCORES)),
                               trace=_trace, **(_trace_kwargs or {}))
    out = np.concatenate([res.results[c]["y"] for c in range(NCORES)], axis=0)
    if _trace:
        _CACHE["last_result"] = res
    return out.astype(f32)
